# revision 7
# baseline (speedup 1.0000x reference)
"""Trainium2 Bass kernel for nn_Attention_40157944217639.

Computes, for B=32, S=256, P=1024:
    q = uf @ W1.T + b1; k = uf @ W2.T + b2; v = uf @ W3.T + b3
    att = log_softmax(tril_mask((q^T k) / S * (g^T d)), axis=-1)
    out = v @ att            # [B, S, P]

Data-parallel over B across 8 NeuronCores (4 batches per core). All
matmuls run as float32r (full-rate fp32 on the PE). The causal mask is
written as real -inf so the final matmul reproduces the reference's
IEEE inf/NaN pattern exactly.
"""

import os
import sys

if "/opt/trn_rl_repo" not in sys.path:
    sys.path.insert(0, "/opt/trn_rl_repo")

import numpy as np

import concourse.bass as bass  # noqa: F401  (engine types via nc)
import concourse.tile as tile
from concourse import bacc, mybir
from concourse.bass_utils import run_bass_kernel_spmd

B, S, P = 32, 256, 1024
NCORES = 8
NB = B // NCORES          # batches per core
PC = P // 128             # p-chunks
F32 = mybir.dt.float32
F32R = mybir.dt.float32r
BF16 = mybir.dt.bfloat16
U8 = mybir.dt.uint8
AF = mybir.ActivationFunctionType
ALU = mybir.AluOpType

_prog_cache = {}


def build_program():
    if "nc" in _prog_cache:
        return _prog_cache["nc"]
    stage = int(os.environ.get("KSTAGE", "3"))

    nc = bacc.Bacc()

    xt_d = nc.declare_dram_parameter("xt", [NB, P, S], F32R, isOutput=False)
    gg_d = nc.declare_dram_parameter("gg", [NB, S, P], F32R, isOutput=False)
    dd_d = nc.declare_dram_parameter("dd", [NB, S, P], F32R, isOutput=False)
    w1_d = nc.declare_dram_parameter("w1t", [P, P], F32R, isOutput=False)
    w2_d = nc.declare_dram_parameter("w2t", [P, P], F32R, isOutput=False)
    w3_d = nc.declare_dram_parameter("w3t", [P, P], F32R, isOutput=False)
    b1_d = nc.declare_dram_parameter("b1b", [128, P], BF16, isOutput=False)
    b2_d = nc.declare_dram_parameter("b2b", [128, P], BF16, isOutput=False)
    b3_d = nc.declare_dram_parameter("b3c", [128, PC], F32, isOutput=False)
    mk_d = nc.declare_dram_parameter("msk", [128, P + 896], U8, isOutput=False)
    out_d = nc.declare_dram_parameter("out", [NB, S, P], F32, isOutput=True)

    with tile.TileContext(nc) as tc:
        with (
            tc.tile_pool(name="wp", bufs=1) as wp,
            tc.tile_pool(name="cp", bufs=1) as cp,
            tc.tile_pool(name="xp", bufs=2) as xp,
            tc.tile_pool(name="gp", bufs=2) as gp,
            tc.tile_pool(name="qp", bufs=1) as qp,
            tc.tile_pool(name="tp", bufs=1) as tp,
            tc.tile_pool(name="ap", bufs=2) as app,
            tc.tile_pool(name="sm", bufs=4) as sm,
            tc.tile_pool(name="ps", bufs=4, space="PSUM") as ps,
        ):
            # ---- persistent weights / constants ----
            ws = []
            for wname, wd in (("w1", w1_d), ("w2", w2_d), ("w3", w3_d)):
                wtile = wp.tile([128, PC * P], F32R, tag=wname)
                # DRAM [P, P] rows (128c + r) -> tile[:, c*P + e]
                nc.sync.dma_start(
                    wtile[:].rearrange("p (c e) -> p c e", e=P),
                    wd.rearrange("(c p) e -> p c e", p=128),
                )
                ws.append(wtile)
            w1s, w2s, w3s = ws

            b1b = cp.tile([128, P], BF16, tag="b1b")
            b2b = cp.tile([128, P], BF16, tag="b2b")
            b3c = cp.tile([128, PC], F32, tag="b3c")
            msk = cp.tile([128, P + 896], U8, tag="msk")
            ninf = cp.tile([128, P], F32, tag="ninf")
            nc.sync.dma_start(b1b[:], b1_d[:])
            nc.sync.dma_start(b2b[:], b2_d[:])
            nc.sync.dma_start(b3c[:], b3_d[:])
            nc.sync.dma_start(msk[:], mk_d[:])
            nc.vector.memset(ninf[:], float("-inf"))

            for b in range(NB):
                # ---- per-batch loads ----
                xts = xp.tile([128, PC * S], F32R, tag="xts")
                nc.sync.dma_start(
                    xts[:].rearrange("p (c s) -> p c s", s=S),
                    xt_d[b].rearrange("(c p) s -> p c s", p=128),
                )
                gt = [gp.tile([128, P], F32R, tag=f"g{sc}", name=f"g{sc}_{b}") for sc in range(2)]
                dt_ = [gp.tile([128, P], F32R, tag=f"d{sc}", name=f"d{sc}_{b}") for sc in range(2)]
                for sc in range(2):
                    nc.sync.dma_start(gt[sc][:], gg_d[b, sc * 128:(sc + 1) * 128, :])
                    nc.sync.dma_start(dt_[sc][:], dd_d[b, sc * 128:(sc + 1) * 128, :])

                # ---- linears: q, k in [s, e]; vT in [e, s] ----
                qt = [qp.tile([128, P], F32R, tag=f"q{sc}", name=f"q{sc}_{b}") for sc in range(2)]
                kt = [qp.tile([128, P], F32R, tag=f"k{sc}", name=f"k{sc}_{b}") for sc in range(2)]
                vts = qp.tile([128, PC * S], F32R, tag="vts")

                for sc in range(2):
                    for h in range(2):
                        pq = ps.tile([128, 512], F32, tag="mm")
                        for ic in range(PC):
                            nc.tensor.matmul(
                                pq[:],
                                xts[:, ic * S + sc * 128: ic * S + sc * 128 + 128],
                                w1s[:, ic * P + h * 512: ic * P + h * 512 + 512],
                                start=(ic == 0), stop=(ic == PC - 1),
                            )
                        nc.vector.tensor_add(
                            qt[sc][:, h * 512:(h + 1) * 512], pq[:],
                            b1b[:, h * 512:(h + 1) * 512],
                        )
                        pk = ps.tile([128, 512], F32, tag="ob")
                        for ic in range(PC):
                            nc.tensor.matmul(
                                pk[:],
                                xts[:, ic * S + sc * 128: ic * S + sc * 128 + 128],
                                w2s[:, ic * P + h * 512: ic * P + h * 512 + 512],
                                start=(ic == 0), stop=(ic == PC - 1),
                            )
                        nc.vector.tensor_add(
                            kt[sc][:, h * 512:(h + 1) * 512], pk[:],
                            b2b[:, h * 512:(h + 1) * 512],
                        )

                for ec in range(PC):
                    pv = ps.tile([128, 512], F32, tag="mm")
                    for ic in range(PC):
                        nc.tensor.matmul(
                            pv[:, :S],
                            w3s[:, ic * P + ec * 128: ic * P + ec * 128 + 128],
                            xts[:, ic * S:(ic + 1) * S],
                            start=(ic == 0), stop=(ic == PC - 1),
                        )
                    nc.vector.tensor_scalar_add(
                        vts[:, ec * S:(ec + 1) * S], pv[:, :S], b3c[:, ec:ec + 1]
                    )

                if stage == 1:
                    for sc in range(2):
                        osb = app.tile([128, P], F32, tag="osb", name=f"osbq{sc}_{b}")
                        nc.vector.tensor_copy(osb[:], qt[sc][:])
                        nc.sync.dma_start(out_d[b, sc * 128:(sc + 1) * 128, :], osb[:])
                    continue

                # ---- attention scores + log_softmax + output ----
                ou = [[ps.tile([128, 512], F32, tag="ob", name=f"ou{sc}{h}_{b}")
                       for h in range(2)] for sc in range(2)]
                for pc in range(PC):
                    pqk = []
                    pg = []
                    for h in range(2):
                        pq_ = ps.tile([128, 512], F32, tag="mm")
                        for sc in range(2):
                            nc.tensor.matmul(
                                pq_[:],
                                qt[sc][:, pc * 128:(pc + 1) * 128],
                                kt[sc][:, h * 512:(h + 1) * 512],
                                start=(sc == 0), stop=(sc == 1),
                            )
                        pqk.append(pq_)
                        pg_ = ps.tile([128, 512], F32, tag="mm")
                        for sc in range(2):
                            nc.tensor.matmul(
                                pg_[:],
                                gt[sc][:, pc * 128:(pc + 1) * 128],
                                dt_[sc][:, h * 512:(h + 1) * 512],
                                start=(sc == 0), stop=(sc == 1),
                            )
                        pg.append(pg_)

                    gsc = tp.tile([128, P], F32, tag="gsc")
                    for h in range(2):
                        nc.scalar.copy(gsc[:, h * 512:(h + 1) * 512], pg[h][:])
                    # t = qk * graph (raw, unscaled); 1/S folded into exp/sub
                    t = tp.tile([128, P], F32, tag="t")
                    for h in range(2):
                        nc.vector.tensor_tensor(
                            out=t[:, h * 512:(h + 1) * 512], in0=pqk[h][:],
                            in1=gsc[:, h * 512:(h + 1) * 512], op=ALU.mult,
                        )
                    off = 896 - 128 * pc
                    nc.vector.copy_predicated(
                        t[:], msk[:, off: off + P], ninf[:]
                    )
                    mx = sm.tile([128, 1], F32, tag="mx")
                    nc.vector.reduce_max(mx[:], t[:], axis=mybir.AxisListType.X)
                    att = app.tile([128, P], F32R, tag="att")
                    negm = sm.tile([128, 1], F32, tag="negm")
                    nc.vector.tensor_scalar_mul(negm[:], mx[:], -1.0 / S)
                    ms = sm.tile([128, 1], F32, tag="ms")
                    nc.vector.tensor_scalar_mul(ms[:], mx[:], 1.0 / S)
                    se = sm.tile([128, 1], F32, tag="se")
                    nc.scalar.activation(
                        att[:], t[:], AF.Exp, bias=negm[:], scale=1.0 / S,
                        accum_out=se[:],
                    )
                    lse = sm.tile([128, 1], F32, tag="lse")
                    nc.scalar.activation(lse[:], se[:], AF.Ln)
                    lset = sm.tile([128, 1], F32, tag="lset")
                    nc.vector.tensor_add(lset[:], lse[:], ms[:])
                    nc.vector.tensor_scalar(
                        out=att[:], in0=t[:], scalar1=1.0 / S, scalar2=lset[:],
                        op0=ALU.mult, op1=ALU.subtract,
                    )

                    if stage >= 3:
                        for sc in range(2):
                            for h in range(2):
                                nc.tensor.matmul(
                                    ou[sc][h][:],
                                    vts[:, pc * S + sc * 128: pc * S + sc * 128 + 128],
                                    att[:, h * 512:(h + 1) * 512],
                                    start=(pc == 0), stop=(pc == PC - 1),
                                )
                    else:
                        if pc == 0:
                            osb = app.tile([128, P], F32, tag="osb", name=f"osba_{b}")
                            nc.vector.tensor_copy(osb[:], att[:])
                            nc.sync.dma_start(out_d[b, 0:128, :], osb[:])

                if stage >= 3:
                    for sc in range(2):
                        osb = app.tile([128, P], F32, tag="osb", name=f"osb{sc}_{b}")
                        for h in range(2):
                            nc.scalar.copy(osb[:, h * 512:(h + 1) * 512], ou[sc][h][:])
                        nc.sync.dma_start(
                            out_d[b, sc * 128:(sc + 1) * 128, :], osb[:]
                        )

    nc.compile()
    _prog_cache["nc"] = nc
    return nc


def kernel(user_feature, global_graph_feature, global_dist_feature,
           W1, b1, W2, b2, W3, b3):
    uf = np.ascontiguousarray(np.asarray(user_feature, dtype=np.float32))
    gg = np.ascontiguousarray(np.asarray(global_graph_feature, dtype=np.float32))
    dd = np.ascontiguousarray(np.asarray(global_dist_feature, dtype=np.float32))
    W1 = np.asarray(W1, dtype=np.float32)
    W2 = np.asarray(W2, dtype=np.float32)
    W3 = np.asarray(W3, dtype=np.float32)
    b1 = np.asarray(b1, dtype=np.float32)
    b2 = np.asarray(b2, dtype=np.float32)
    b3 = np.asarray(b3, dtype=np.float32)

    nc = build_program()

    xt = np.ascontiguousarray(np.swapaxes(uf, 1, 2))          # [B, P, S]
    w1t = np.ascontiguousarray(W1.T)
    w2t = np.ascontiguousarray(W2.T)
    w3t = np.ascontiguousarray(W3.T)
    import ml_dtypes
    b1b = np.broadcast_to(b1, (128, P)).astype(ml_dtypes.bfloat16)
    b2b = np.broadcast_to(b2, (128, P)).astype(ml_dtypes.bfloat16)
    b3c = np.ascontiguousarray(b3.reshape(PC, 128).T)          # [128, PC]
    u = np.arange(P + 896)[None, :] - 896
    r = np.arange(128)[:, None]
    msk = (u > r).astype(np.uint8)                             # 1 = masked

    in_maps = []
    for c in range(NCORES):
        sl = slice(c * NB, (c + 1) * NB)
        in_maps.append({
            "xt": xt[sl], "gg": gg[sl], "dd": dd[sl],
            "w1t": w1t, "w2t": w2t, "w3t": w3t,
            "b1b": b1b, "b2b": b2b, "b3c": b3c, "msk": msk,
        })

    res = run_bass_kernel_spmd(nc, in_maps, list(range(NCORES)))
    out = np.empty((B, S, P), np.float32)
    for c in range(NCORES):
        out[c * NB:(c + 1) * NB] = res.results[c]["out"]
    return out


# revision 11
# speedup vs baseline: 1.5093x; 1.5093x over previous
"""Trainium2 Bass kernel for nn_Attention_40157944217639.

Computes, for B=32, S=256, P=1024:
    q = uf @ W1.T + b1; k = uf @ W2.T + b2; v = uf @ W3.T + b3
    att = log_softmax(tril_mask((q^T k) / S * (g^T d)), axis=-1)
    out = v @ att            # [B, S, P]

Data-parallel over B across 8 NeuronCores (4 batches per core). All
matmuls run as float32r (full-rate fp32 on the PE). The causal mask is
real -inf so the final matmul reproduces the reference's IEEE inf/NaN
pattern.

Structure notes:
- Triangular saving: per 128-row score chunk pc only columns
  [0, 128*(pc+1)) are computed; the rest is a -inf memset. The second
  512-col output piece only accumulates from chunk 3 on.
- log_softmax row shift: att holds t/S with -inf mask; the per-row
  constant c[p] = max/S + ln(sum exp) is applied as a rank-1 correction
  out -= (v @ c) 1^T at the final PSUM->SBUF copy, so ACT runs only Exp
  inside the chunk loop (one Ln per batch; no activation-table thrash).
- Chunk pc's qk/graph matmuls are emitted before chunk pc-1's out
  matmuls so the PE never idles on the softmax chain.
"""

import sys

if "/opt/trn_rl_repo" not in sys.path:
    sys.path.insert(0, "/opt/trn_rl_repo")

import numpy as np

import concourse.tile as tile
from concourse import bacc, mybir
from concourse.bass_utils import run_bass_kernel_spmd

B, S, P = 32, 256, 1024
NCORES = 8
NB = B // NCORES          # batches per core
PC = P // 128             # p-chunks
F32 = mybir.dt.float32
F32R = mybir.dt.float32r
BF16 = mybir.dt.bfloat16
U8 = mybir.dt.uint8
AF = mybir.ActivationFunctionType
ALU = mybir.AluOpType
NEG_INF = float("-inf")

_prog_cache = {}


def build_program():
    if "nc" in _prog_cache:
        return _prog_cache["nc"]

    nc = bacc.Bacc()

    xt_d = nc.declare_dram_parameter("xt", [NB, P, S], F32R, isOutput=False)
    gg_d = nc.declare_dram_parameter("gg", [NB, S, P], F32R, isOutput=False)
    dd_d = nc.declare_dram_parameter("dd", [NB, S, P], F32R, isOutput=False)
    w1_d = nc.declare_dram_parameter("w1t", [P, P], F32R, isOutput=False)
    w2_d = nc.declare_dram_parameter("w2t", [P, P], F32R, isOutput=False)
    w3_d = nc.declare_dram_parameter("w3t", [P, P], F32R, isOutput=False)
    b1_d = nc.declare_dram_parameter("b1b", [128, P], BF16, isOutput=False)
    b2_d = nc.declare_dram_parameter("b2b", [128, P], BF16, isOutput=False)
    b3_d = nc.declare_dram_parameter("b3c", [128, PC], F32, isOutput=False)
    mk_d = nc.declare_dram_parameter("msk", [128, 128], U8, isOutput=False)
    out_d = nc.declare_dram_parameter("out", [NB, S, P], F32, isOutput=True)

    with tile.TileContext(nc) as tc:
        with (
            tc.tile_pool(name="wp", bufs=1) as wp,
            tc.tile_pool(name="cp", bufs=1) as cp,
            tc.tile_pool(name="xp", bufs=2) as xp,
            tc.tile_pool(name="gp", bufs=2) as gp,
            tc.tile_pool(name="qp", bufs=1) as qp,
            tc.tile_pool(name="tp", bufs=2) as tp,
            tc.tile_pool(name="ap", bufs=2) as app,
            tc.tile_pool(name="sm", bufs=2) as sm,
            tc.tile_pool(name="ps", bufs=4, space="PSUM") as ps,
        ):
            # ---- persistent weights / constants ----
            ws = []
            for wname, wd in (("w1", w1_d), ("w2", w2_d), ("w3", w3_d)):
                wtile = wp.tile([128, PC * P], F32R, tag=wname, name=wname + "s")
                nc.sync.dma_start(
                    wtile[:].rearrange("p (c e) -> p c e", e=P),
                    wd.rearrange("(c p) e -> p c e", p=128),
                )
                ws.append(wtile)
            w1s, w2s, w3s = ws

            b1b = cp.tile([128, P], BF16, tag="b1b")
            b2b = cp.tile([128, P], BF16, tag="b2b")
            b3c = cp.tile([128, PC], F32, tag="b3c")
            msk = cp.tile([128, 128], U8, tag="msk")
            ninf = cp.tile([128, 512], F32, tag="ninf")
            nc.sync.dma_start(b1b[:], b1_d[:])
            nc.sync.dma_start(b2b[:], b2_d[:])
            nc.sync.dma_start(b3c[:], b3_d[:])
            nc.sync.dma_start(msk[:], mk_d[:])
            nc.vector.memset(ninf[:], NEG_INF)

            for b in range(NB):
                # ---- per-batch loads ----
                xts = xp.tile([128, PC * S], F32R, tag="xts", name=f"xts_{b}")
                nc.sync.dma_start(
                    xts[:].rearrange("p (c s) -> p c s", s=S),
                    xt_d[b].rearrange("(c p) s -> p c s", p=128),
                )
                gt = [gp.tile([128, P], F32R, tag=f"g{sc}", name=f"g{sc}_{b}")
                      for sc in range(2)]
                dt_ = [gp.tile([128, P], F32R, tag=f"d{sc}", name=f"d{sc}_{b}")
                       for sc in range(2)]
                for sc in range(2):
                    nc.sync.dma_start(gt[sc][:], gg_d[b, sc * 128:(sc + 1) * 128, :])
                    nc.sync.dma_start(dt_[sc][:], dd_d[b, sc * 128:(sc + 1) * 128, :])

                # ---- linears: q, k in [s, e]; vT in [e, s] ----
                qt = [qp.tile([128, P], F32R, tag=f"q{sc}", name=f"q{sc}_{b}")
                      for sc in range(2)]
                kt = [qp.tile([128, P], F32R, tag=f"k{sc}", name=f"k{sc}_{b}")
                      for sc in range(2)]
                vts = qp.tile([128, PC * S], F32R, tag="vts", name=f"vts_{b}")

                for sc in range(2):
                    for h in range(2):
                        pq = ps.tile([128, 512], F32, tag="mm",
                                     name=f"pq{sc}{h}_{b}")
                        for ic in range(PC):
                            nc.tensor.matmul(
                                pq[:],
                                xts[:, ic * S + sc * 128: ic * S + sc * 128 + 128],
                                w1s[:, ic * P + h * 512: ic * P + h * 512 + 512],
                                start=(ic == 0), stop=(ic == PC - 1),
                            )
                        nc.vector.tensor_add(
                            qt[sc][:, h * 512:(h + 1) * 512], pq[:],
                            b1b[:, h * 512:(h + 1) * 512],
                        )
                        pk = ps.tile([128, 512], F32, tag="ob",
                                     name=f"pk{sc}{h}_{b}")
                        for ic in range(PC):
                            nc.tensor.matmul(
                                pk[:],
                                xts[:, ic * S + sc * 128: ic * S + sc * 128 + 128],
                                w2s[:, ic * P + h * 512: ic * P + h * 512 + 512],
                                start=(ic == 0), stop=(ic == PC - 1),
                            )
                        nc.vector.tensor_add(
                            kt[sc][:, h * 512:(h + 1) * 512], pk[:],
                            b2b[:, h * 512:(h + 1) * 512],
                        )

                for ec in range(PC):
                    pv = ps.tile([128, 512], F32, tag="mm", name=f"pv{ec}_{b}")
                    for ic in range(PC):
                        nc.tensor.matmul(
                            pv[:, :S],
                            w3s[:, ic * P + ec * 128: ic * P + ec * 128 + 128],
                            xts[:, ic * S:(ic + 1) * S],
                            start=(ic == 0), stop=(ic == PC - 1),
                        )
                    nc.vector.tensor_scalar_add(
                        vts[:, ec * S:(ec + 1) * S], pv[:, :S], b3c[:, ec:ec + 1]
                    )

                # ---- attention scores + log_softmax + output ----
                se_all = sm.tile([128, PC], F32, tag="se_all", name=f"sea_{b}")
                m_all = sm.tile([128, PC], F32, tag="m_all", name=f"ma_{b}")
                ou = [[None, None], [None, None]]
                for sc in range(2):
                    ou[sc][0] = ps.tile([128, 512], F32, tag="ob",
                                        name=f"ou{sc}0_{b}")

                def emit_out_mms(pc, att):
                    for sc in range(2):
                        if pc == 3:
                            ou[sc][1] = ps.tile([128, 512], F32, tag="ob",
                                                name=f"ou{sc}1_{b}")
                        for h in range(2):
                            if h == 1 and pc < 3:
                                continue
                            nc.tensor.matmul(
                                ou[sc][h][:],
                                vts[:, pc * S + sc * 128: pc * S + sc * 128 + 128],
                                att[:, h * 512:(h + 1) * 512],
                                start=(pc == (0 if h == 0 else 3)),
                                stop=(pc == PC - 1),
                            )

                prev = None  # (pc, att) awaiting out-matmuls
                for pc in range(PC):
                    wc = 128 * (pc + 1)          # computed width
                    wr = 512 if pc < 3 else P    # rounded (written) width
                    # score matmuls for this chunk (PE-dense, ahead of the
                    # previous chunk's out matmuls)
                    pqk = []
                    pg = []
                    for h in range(2):
                        a = h * 512
                        n = min(wc, 512 * (h + 1)) - a
                        if n <= 0:
                            pqk.append(None)
                            pg.append(None)
                            continue
                        pq_ = ps.tile([128, 512], F32, tag="mm",
                                      name=f"pqk{h}_{b}_{pc}")
                        for sc in range(2):
                            nc.tensor.matmul(
                                pq_[:, :n],
                                qt[sc][:, pc * 128:(pc + 1) * 128],
                                kt[sc][:, a: a + n],
                                start=(sc == 0), stop=(sc == 1),
                            )
                        pqk.append(pq_)
                        pg_ = ps.tile([128, 512], F32, tag="mm",
                                      name=f"pg{h}_{b}_{pc}")
                        for sc in range(2):
                            nc.tensor.matmul(
                                pg_[:, :n],
                                gt[sc][:, pc * 128:(pc + 1) * 128],
                                dt_[sc][:, a: a + n],
                                start=(sc == 0), stop=(sc == 1),
                            )
                        pg.append(pg_)

                    if prev is not None:
                        emit_out_mms(*prev)

                    # t = qk * graph over [0, wc)
                    t = tp.tile([128, P], F32, tag="t", name=f"t_{b}_{pc}")
                    for h in range(2):
                        if pqk[h] is None:
                            continue
                        a = h * 512
                        n = min(wc, 512 * (h + 1)) - a
                        nc.vector.tensor_copy(t[:, a: a + n], pqk[h][:, :n])
                        nc.vector.tensor_tensor(
                            out=t[:, a: a + n], in0=t[:, a: a + n],
                            in1=pg[h][:, :n], op=ALU.mult,
                        )
                    # causal mask on the diagonal 128-block
                    nc.vector.copy_predicated(
                        t[:, pc * 128: pc * 128 + 128], msk[:], ninf[:, 0:128]
                    )
                    mxc = m_all[:, pc: pc + 1]
                    nc.vector.reduce_max(mxc, t[:, 0:wc],
                                         axis=mybir.AxisListType.X)
                    negm = sm.tile([128, 1], F32, tag="negm",
                                   name=f"negm_{b}_{pc}")
                    nc.vector.tensor_scalar_mul(negm[:], mxc, -1.0 / S)
                    att = app.tile([128, P], F32R, tag="att",
                                   name=f"att_{b}_{pc}")
                    # exp for the row sums only (att is scratch here)
                    nc.scalar.activation(
                        att[:, 0:wc], t[:, 0:wc], AF.Exp, bias=negm[:],
                        scale=1.0 / S, accum_out=se_all[:, pc: pc + 1],
                    )
                    # att = t/S (finite part); -inf tail
                    nc.vector.tensor_scalar_mul(att[:, 0:wc], t[:, 0:wc],
                                                1.0 / S)
                    if wr > wc:
                        nc.vector.tensor_copy(att[:, wc:wr], ninf[:, 0:wr - wc])
                    prev = (pc, att)

                emit_out_mms(*prev)

                # ---- per-row constant c = m/S + ln(se); rank-1 fold ----
                lna = sm.tile([128, PC], F32, tag="lna", name=f"lna_{b}")
                nc.scalar.activation(lna[:], se_all[:], AF.Ln)
                c_all = sm.tile([128, PC], F32R, tag="c_all", name=f"ca_{b}")
                nc.vector.tensor_scalar(
                    out=c_all[:], in0=m_all[:], scalar1=1.0 / S, scalar2=None,
                    op0=ALU.mult,
                )
                nc.vector.tensor_tensor(out=c_all[:], in0=c_all[:], in1=lna[:],
                                        op=ALU.add)
                # fp32r matmul needs even moving-free-dim: duplicate c into
                # even/odd columns of c2, use [*, 2] slices (col 1 unused)
                c2 = sm.tile([128, 2 * PC], F32R, tag="c2", name=f"c2_{b}")
                c2v = c2[:].rearrange("p (a two) -> p two a", two=2)
                nc.vector.tensor_copy(c2v[:, 0, :], c_all[:])
                nc.vector.tensor_copy(c2v[:, 1, :], c_all[:])
                vcs = sm.tile([128, 2], F32, tag="vcs", name=f"vcs_{b}")
                for sc in range(2):
                    pvc = ps.tile([128, 512], F32, tag="mm",
                                  name=f"pvc{sc}_{b}")
                    for pc in range(PC):
                        nc.tensor.matmul(
                            pvc[:, 0:2],
                            vts[:, pc * S + sc * 128: pc * S + sc * 128 + 128],
                            c2[:, 2 * pc: 2 * pc + 2],
                            start=(pc == 0), stop=(pc == PC - 1),
                        )
                    nc.vector.tensor_copy(vcs[:, sc: sc + 1], pvc[:, 0:1])

                for sc in range(2):
                    osb = app.tile([128, P], F32, tag="osb", name=f"osb{sc}_{b}")
                    for h in range(2):
                        nc.vector.tensor_scalar_sub(
                            osb[:, h * 512:(h + 1) * 512], ou[sc][h][:],
                            vcs[:, sc: sc + 1],
                        )
                    nc.sync.dma_start(
                        out_d[b, sc * 128:(sc + 1) * 128, :], osb[:]
                    )

    nc.compile()
    _prog_cache["nc"] = nc
    return nc


def kernel(user_feature, global_graph_feature, global_dist_feature,
           W1, b1, W2, b2, W3, b3):
    uf = np.ascontiguousarray(np.asarray(user_feature, dtype=np.float32))
    gg = np.ascontiguousarray(np.asarray(global_graph_feature, dtype=np.float32))
    dd = np.ascontiguousarray(np.asarray(global_dist_feature, dtype=np.float32))
    W1 = np.asarray(W1, dtype=np.float32)
    W2 = np.asarray(W2, dtype=np.float32)
    W3 = np.asarray(W3, dtype=np.float32)
    b1 = np.asarray(b1, dtype=np.float32)
    b2 = np.asarray(b2, dtype=np.float32)
    b3 = np.asarray(b3, dtype=np.float32)

    nc = build_program()

    xt = np.ascontiguousarray(np.swapaxes(uf, 1, 2))          # [B, P, S]
    w1t = np.ascontiguousarray(W1.T)
    w2t = np.ascontiguousarray(W2.T)
    w3t = np.ascontiguousarray(W3.T)
    import ml_dtypes
    b1b = np.broadcast_to(b1, (128, P)).astype(ml_dtypes.bfloat16)
    b2b = np.broadcast_to(b2, (128, P)).astype(ml_dtypes.bfloat16)
    b3c = np.ascontiguousarray(b3.reshape(PC, 128).T)          # [128, PC]
    j = np.arange(128)[None, :]
    r = np.arange(128)[:, None]
    msk = (j > r).astype(np.uint8)                             # diag block mask

    in_maps = []
    for c in range(NCORES):
        sl = slice(c * NB, (c + 1) * NB)
        in_maps.append({
            "xt": xt[sl], "gg": gg[sl], "dd": dd[sl],
            "w1t": w1t, "w2t": w2t, "w3t": w3t,
            "b1b": b1b, "b2b": b2b, "b3c": b3c, "msk": msk,
        })

    res = run_bass_kernel_spmd(nc, in_maps, list(range(NCORES)))
    out = np.empty((B, S, P), np.float32)
    for c in range(NCORES):
        out[c * NB:(c + 1) * NB] = res.results[c]["out"]
    return out


# revision 12
# speedup vs baseline: 1.8022x; 1.1941x over previous
"""Trainium2 Bass kernel for nn_Attention_40157944217639.

Computes, for B=32, S=256, P=1024:
    q = uf @ W1.T + b1; k = uf @ W2.T + b2; v = uf @ W3.T + b3
    att = log_softmax(tril_mask((q^T k) / S * (g^T d)), axis=-1)
    out = v @ att            # [B, S, P]

Data-parallel over B across 8 NeuronCores (4 batches per core). All
matmuls run as float32r (full-rate fp32 on the PE). The causal mask is
real -inf so the final matmul reproduces the reference's IEEE inf/NaN
pattern.

Structure notes:
- Triangular saving: per 128-row score chunk pc only columns
  [0, 128*(pc+1)) are computed; the rest is a -inf memset. The second
  512-col output piece only accumulates from chunk 3 on.
- log_softmax row shift: att holds t/S with -inf mask; the per-row
  constant c[p] = max/S + ln(sum exp) is applied as a rank-1 correction
  out -= (v @ c) 1^T at the final PSUM->SBUF copy, so ACT runs only Exp
  inside the chunk loop (one Ln per batch; no activation-table thrash).
- Chunk pc's qk/graph matmuls are emitted before chunk pc-1's out
  matmuls so the PE never idles on the softmax chain.
"""

import sys

if "/opt/trn_rl_repo" not in sys.path:
    sys.path.insert(0, "/opt/trn_rl_repo")

import numpy as np

import concourse.tile as tile
from concourse import bacc, mybir
from concourse.bass_utils import run_bass_kernel_spmd

B, S, P = 32, 256, 1024
NCORES = 8
NB = B // NCORES          # batches per core
PC = P // 128             # p-chunks
F32 = mybir.dt.float32
F32R = mybir.dt.float32r
BF16 = mybir.dt.bfloat16
U8 = mybir.dt.uint8
AF = mybir.ActivationFunctionType
ALU = mybir.AluOpType
NEG_INF = float("-inf")

_prog_cache = {}


def build_program():
    if "nc" in _prog_cache:
        return _prog_cache["nc"]

    nc = bacc.Bacc()

    xt_d = nc.declare_dram_parameter("xt", [NB, P, S], F32R, isOutput=False)
    gg_d = nc.declare_dram_parameter("gg", [NB, S, P], F32R, isOutput=False)
    dd_d = nc.declare_dram_parameter("dd", [NB, S, P], F32R, isOutput=False)
    w1_d = nc.declare_dram_parameter("w1t", [P, P], F32R, isOutput=False)
    w2_d = nc.declare_dram_parameter("w2t", [P, P], F32R, isOutput=False)
    w3_d = nc.declare_dram_parameter("w3t", [P, P], F32R, isOutput=False)
    b1_d = nc.declare_dram_parameter("b1b", [128, P], BF16, isOutput=False)
    b2_d = nc.declare_dram_parameter("b2b", [128, P], BF16, isOutput=False)
    b3_d = nc.declare_dram_parameter("b3c", [128, PC], F32, isOutput=False)
    mk_d = nc.declare_dram_parameter("msk", [128, 128], U8, isOutput=False)
    out_d = nc.declare_dram_parameter("out", [NB, S, P], F32, isOutput=True)

    with tile.TileContext(nc) as tc:
        with (
            tc.tile_pool(name="wp", bufs=1) as wp,
            tc.tile_pool(name="cp", bufs=1) as cp,
            tc.tile_pool(name="xp", bufs=2) as xp,
            tc.tile_pool(name="gp", bufs=2) as gp,
            tc.tile_pool(name="qp", bufs=1) as qp,
            tc.tile_pool(name="tp", bufs=2) as tp,
            tc.tile_pool(name="ap", bufs=2) as app,
            tc.tile_pool(name="sm", bufs=2) as sm,
            tc.tile_pool(name="ps", bufs=4, space="PSUM") as ps,
        ):
            # ---- persistent weights / constants ----
            ws = []
            for wname, wd in (("w1", w1_d), ("w2", w2_d), ("w3", w3_d)):
                wtile = wp.tile([128, PC * P], F32R, tag=wname, name=wname + "s")
                nc.sync.dma_start(
                    wtile[:].rearrange("p (c e) -> p c e", e=P),
                    wd.rearrange("(c p) e -> p c e", p=128),
                )
                ws.append(wtile)
            w1s, w2s, w3s = ws

            b1b = cp.tile([128, P], BF16, tag="b1b")
            b2b = cp.tile([128, P], BF16, tag="b2b")
            b3c = cp.tile([128, PC], F32, tag="b3c")
            msk = cp.tile([128, 128], U8, tag="msk")
            ninf = cp.tile([128, 512], F32, tag="ninf")
            nc.sync.dma_start(b1b[:], b1_d[:])
            nc.sync.dma_start(b2b[:], b2_d[:])
            nc.sync.dma_start(b3c[:], b3_d[:])
            nc.sync.dma_start(msk[:], mk_d[:])
            nc.vector.memset(ninf[:], NEG_INF)

            pending_finalize = []
            for b in range(NB):
                # ---- per-batch loads ----
                xts = xp.tile([128, PC * S], F32R, tag="xts", name=f"xts_{b}")
                nc.sync.dma_start(
                    xts[:].rearrange("p (c s) -> p c s", s=S),
                    xt_d[b].rearrange("(c p) s -> p c s", p=128),
                )
                gt = [gp.tile([128, P], F32R, tag=f"g{sc}", name=f"g{sc}_{b}")
                      for sc in range(2)]
                dt_ = [gp.tile([128, P], F32R, tag=f"d{sc}", name=f"d{sc}_{b}")
                       for sc in range(2)]
                for sc in range(2):
                    nc.sync.dma_start(gt[sc][:], gg_d[b, sc * 128:(sc + 1) * 128, :])
                    nc.sync.dma_start(dt_[sc][:], dd_d[b, sc * 128:(sc + 1) * 128, :])

                # ---- linears: q, k in [s, e]; vT in [e, s] ----
                qt = [qp.tile([128, P], F32R, tag=f"q{sc}", name=f"q{sc}_{b}")
                      for sc in range(2)]
                kt = [qp.tile([128, P], F32R, tag=f"k{sc}", name=f"k{sc}_{b}")
                      for sc in range(2)]
                vts = qp.tile([128, PC * S], F32R, tag="vts", name=f"vts_{b}")

                for sc in range(2):
                    for h in range(2):
                        pq = ps.tile([128, 512], F32, tag="mm",
                                     name=f"pq{sc}{h}_{b}")
                        for ic in range(PC):
                            nc.tensor.matmul(
                                pq[:],
                                xts[:, ic * S + sc * 128: ic * S + sc * 128 + 128],
                                w1s[:, ic * P + h * 512: ic * P + h * 512 + 512],
                                start=(ic == 0), stop=(ic == PC - 1),
                            )
                        nc.vector.tensor_add(
                            qt[sc][:, h * 512:(h + 1) * 512], pq[:],
                            b1b[:, h * 512:(h + 1) * 512],
                        )
                        pk = ps.tile([128, 512], F32, tag="ob",
                                     name=f"pk{sc}{h}_{b}")
                        for ic in range(PC):
                            nc.tensor.matmul(
                                pk[:],
                                xts[:, ic * S + sc * 128: ic * S + sc * 128 + 128],
                                w2s[:, ic * P + h * 512: ic * P + h * 512 + 512],
                                start=(ic == 0), stop=(ic == PC - 1),
                            )
                        nc.vector.tensor_add(
                            kt[sc][:, h * 512:(h + 1) * 512], pk[:],
                            b2b[:, h * 512:(h + 1) * 512],
                        )

                if pending_finalize:
                    pending_finalize.pop(0)()

                for ec in range(PC):
                    pv = ps.tile([128, 512], F32, tag="mm", name=f"pv{ec}_{b}")
                    for ic in range(PC):
                        nc.tensor.matmul(
                            pv[:, :S],
                            w3s[:, ic * P + ec * 128: ic * P + ec * 128 + 128],
                            xts[:, ic * S:(ic + 1) * S],
                            start=(ic == 0), stop=(ic == PC - 1),
                        )
                    nc.vector.tensor_scalar_add(
                        vts[:, ec * S:(ec + 1) * S], pv[:, :S], b3c[:, ec:ec + 1]
                    )

                # ---- attention scores + log_softmax + output ----
                se_all = sm.tile([128, PC], F32, tag="se_all", name=f"sea_{b}")
                m_all = sm.tile([128, PC], F32, tag="m_all", name=f"ma_{b}")
                ou = [[None, None], [None, None]]
                for sc in range(2):
                    ou[sc][0] = ps.tile([128, 512], F32, tag="ob",
                                        name=f"ou{sc}0_{b}")

                def emit_out_mms(pc, att):
                    for sc in range(2):
                        if pc == 3:
                            ou[sc][1] = ps.tile([128, 512], F32, tag="ob",
                                                name=f"ou{sc}1_{b}")
                        for h in range(2):
                            if h == 1 and pc < 3:
                                continue
                            nc.tensor.matmul(
                                ou[sc][h][:],
                                vts[:, pc * S + sc * 128: pc * S + sc * 128 + 128],
                                att[:, h * 512:(h + 1) * 512],
                                start=(pc == (0 if h == 0 else 3)),
                                stop=(pc == PC - 1),
                            )

                prev = None  # (pc, att) awaiting out-matmuls
                for pc in range(PC):
                    wc = 128 * (pc + 1)          # computed width
                    wr = 512 if pc < 3 else P    # rounded (written) width
                    # score matmuls for this chunk (PE-dense, ahead of the
                    # previous chunk's out matmuls)
                    pqk = []
                    pg = []
                    for h in range(2):
                        a = h * 512
                        n = min(wc, 512 * (h + 1)) - a
                        if n <= 0:
                            pqk.append(None)
                            pg.append(None)
                            continue
                        pq_ = ps.tile([128, 512], F32, tag="mm",
                                      name=f"pqk{h}_{b}_{pc}")
                        for sc in range(2):
                            nc.tensor.matmul(
                                pq_[:, :n],
                                qt[sc][:, pc * 128:(pc + 1) * 128],
                                kt[sc][:, a: a + n],
                                start=(sc == 0), stop=(sc == 1),
                            )
                        pqk.append(pq_)
                        pg_ = ps.tile([128, 512], F32, tag="mm",
                                      name=f"pg{h}_{b}_{pc}")
                        for sc in range(2):
                            nc.tensor.matmul(
                                pg_[:, :n],
                                gt[sc][:, pc * 128:(pc + 1) * 128],
                                dt_[sc][:, a: a + n],
                                start=(sc == 0), stop=(sc == 1),
                            )
                        pg.append(pg_)

                    if prev is not None:
                        emit_out_mms(*prev)

                    # t = qk * graph over [0, wc); graph staged via ACT copy
                    t = tp.tile([128, P], F32, tag="t", name=f"t_{b}_{pc}")
                    gsc = tp.tile([128, P], F32, tag="gsc", name=f"gsc_{b}_{pc}")
                    for h in range(2):
                        if pqk[h] is None:
                            continue
                        a = h * 512
                        n = min(wc, 512 * (h + 1)) - a
                        nc.scalar.copy(gsc[:, a: a + n], pg[h][:, :n])
                        nc.vector.tensor_tensor(
                            out=t[:, a: a + n], in0=pqk[h][:, :n],
                            in1=gsc[:, a: a + n], op=ALU.mult,
                        )
                    # causal mask on the diagonal 128-block
                    nc.vector.copy_predicated(
                        t[:, pc * 128: pc * 128 + 128], msk[:], ninf[:, 0:128]
                    )
                    mxc = m_all[:, pc: pc + 1]
                    nc.vector.reduce_max(mxc, t[:, 0:wc],
                                         axis=mybir.AxisListType.X)
                    negm = sm.tile([128, 1], F32, tag="negm",
                                   name=f"negm_{b}_{pc}")
                    nc.vector.tensor_scalar_mul(negm[:], mxc, -1.0 / S)
                    att = app.tile([128, P], F32R, tag="att",
                                   name=f"att_{b}_{pc}")
                    # exp for the row sums only (att is scratch here)
                    nc.scalar.activation(
                        att[:, 0:wc], t[:, 0:wc], AF.Exp, bias=negm[:],
                        scale=1.0 / S, accum_out=se_all[:, pc: pc + 1],
                    )
                    # att = t/S (finite part); -inf tail
                    nc.vector.tensor_scalar_mul(att[:, 0:wc], t[:, 0:wc],
                                                1.0 / S)
                    if wr > wc:
                        nc.vector.tensor_copy(att[:, wc:wr], ninf[:, 0:wr - wc])
                    prev = (pc, att)

                emit_out_mms(*prev)

                # ---- deferred finalize: c = m/S + ln(se); rank-1 fold ----
                def make_finalize(b, se_all, m_all, vts, ou):
                    def finalize():
                        lna = sm.tile([128, PC], F32, tag="lna",
                                      name=f"lna_{b}")
                        nc.scalar.activation(lna[:], se_all[:], AF.Ln)
                        c_all = sm.tile([128, PC], F32R, tag="c_all",
                                        name=f"ca_{b}")
                        nc.vector.tensor_scalar(
                            out=c_all[:], in0=m_all[:], scalar1=1.0 / S,
                            scalar2=None, op0=ALU.mult,
                        )
                        nc.vector.tensor_tensor(out=c_all[:], in0=c_all[:],
                                                in1=lna[:], op=ALU.add)
                        # fp32r matmul needs even moving-free-dim: duplicate
                        # c into even/odd cols of c2 (col 1 result unused)
                        c2 = sm.tile([128, 2 * PC], F32R, tag="c2",
                                     name=f"c2_{b}")
                        c2v = c2[:].rearrange("p (a two) -> p two a", two=2)
                        nc.vector.tensor_copy(c2v[:, 0, :], c_all[:])
                        nc.vector.tensor_copy(c2v[:, 1, :], c_all[:])
                        vcs = sm.tile([128, 2], F32, tag="vcs",
                                      name=f"vcs_{b}")
                        for sc in range(2):
                            pvc = ps.tile([128, 512], F32, tag="mm",
                                          name=f"pvc{sc}_{b}")
                            for pc in range(PC):
                                nc.tensor.matmul(
                                    pvc[:, 0:2],
                                    vts[:, pc * S + sc * 128:
                                        pc * S + sc * 128 + 128],
                                    c2[:, 2 * pc: 2 * pc + 2],
                                    start=(pc == 0), stop=(pc == PC - 1),
                                )
                            nc.vector.tensor_copy(vcs[:, sc: sc + 1],
                                                  pvc[:, 0:1])
                        for sc in range(2):
                            osb = app.tile([128, P], F32, tag="osb",
                                           name=f"osb{sc}_{b}")
                            for h in range(2):
                                nc.vector.tensor_scalar_sub(
                                    osb[:, h * 512:(h + 1) * 512],
                                    ou[sc][h][:], vcs[:, sc: sc + 1],
                                )
                            nc.sync.dma_start(
                                out_d[b, sc * 128:(sc + 1) * 128, :], osb[:]
                            )
                    return finalize

                pending_finalize.append(make_finalize(b, se_all, m_all, vts, ou))

            while pending_finalize:
                pending_finalize.pop(0)()

    nc.compile()
    _prog_cache["nc"] = nc
    return nc


def kernel(user_feature, global_graph_feature, global_dist_feature,
           W1, b1, W2, b2, W3, b3):
    uf = np.ascontiguousarray(np.asarray(user_feature, dtype=np.float32))
    gg = np.ascontiguousarray(np.asarray(global_graph_feature, dtype=np.float32))
    dd = np.ascontiguousarray(np.asarray(global_dist_feature, dtype=np.float32))
    W1 = np.asarray(W1, dtype=np.float32)
    W2 = np.asarray(W2, dtype=np.float32)
    W3 = np.asarray(W3, dtype=np.float32)
    b1 = np.asarray(b1, dtype=np.float32)
    b2 = np.asarray(b2, dtype=np.float32)
    b3 = np.asarray(b3, dtype=np.float32)

    nc = build_program()

    xt = np.ascontiguousarray(np.swapaxes(uf, 1, 2))          # [B, P, S]
    w1t = np.ascontiguousarray(W1.T)
    w2t = np.ascontiguousarray(W2.T)
    w3t = np.ascontiguousarray(W3.T)
    import ml_dtypes
    b1b = np.broadcast_to(b1, (128, P)).astype(ml_dtypes.bfloat16)
    b2b = np.broadcast_to(b2, (128, P)).astype(ml_dtypes.bfloat16)
    b3c = np.ascontiguousarray(b3.reshape(PC, 128).T)          # [128, PC]
    j = np.arange(128)[None, :]
    r = np.arange(128)[:, None]
    msk = (j > r).astype(np.uint8)                             # diag block mask

    in_maps = []
    for c in range(NCORES):
        sl = slice(c * NB, (c + 1) * NB)
        in_maps.append({
            "xt": xt[sl], "gg": gg[sl], "dd": dd[sl],
            "w1t": w1t, "w2t": w2t, "w3t": w3t,
            "b1b": b1b, "b2b": b2b, "b3c": b3c, "msk": msk,
        })

    res = run_bass_kernel_spmd(nc, in_maps, list(range(NCORES)))
    out = np.empty((B, S, P), np.float32)
    for c in range(NCORES):
        out[c * NB:(c + 1) * NB] = res.results[c]["out"]
    return out


# revision 33
# speedup vs baseline: 1.9148x; 1.0625x over previous
"""Trainium2 Bass kernel for nn_Attention_40157944217639.

Computes, for B=32, S=256, P=1024:
    q = uf @ W1.T + b1; k = uf @ W2.T + b2; v = uf @ W3.T + b3
    att = log_softmax(tril_mask((q^T k) / S * (g^T d)), axis=-1)
    out = v @ att            # [B, S, P]

Data-parallel over B across 8 NeuronCores (4 batches per core). All
matmuls run as float32r (full-rate fp32 on the PE). The causal mask is
real -inf so the final matmul reproduces the reference's IEEE inf/NaN
pattern.

Structure notes:
- Triangular saving: per 128-row score chunk pc only columns
  [0, 128*(pc+1)) are computed; the rest is a -inf memset. The second
  512-col output piece only accumulates from chunk 3 on.
- log_softmax row shift: att holds t/S with -inf mask; the per-row
  constant c[p] = max/S + ln(sum exp) is applied as a rank-1 correction
  out -= (v @ c) 1^T at the final PSUM->SBUF copy, so ACT runs only Exp
  inside the chunk loop (one Ln per batch; no activation-table thrash).
- Chunk pc's qk/graph matmuls are emitted before chunk pc-1's out
  matmuls so the PE never idles on the softmax chain.
"""

import sys

if "/opt/trn_rl_repo" not in sys.path:
    sys.path.insert(0, "/opt/trn_rl_repo")

import numpy as np

import concourse.tile as tile
from concourse import bacc, mybir
from concourse.bass_utils import run_bass_kernel_spmd

B, S, P = 32, 256, 1024
NCORES = 8
NB = B // NCORES          # batches per core
PC = P // 128             # p-chunks
F32 = mybir.dt.float32
F32R = mybir.dt.float32r
BF16 = mybir.dt.bfloat16
U8 = mybir.dt.uint8
AF = mybir.ActivationFunctionType
ALU = mybir.AluOpType
NEG_INF = float("-inf")

_prog_cache = {}


def build_program():
    if "nc" in _prog_cache:
        return _prog_cache["nc"]

    nc = bacc.Bacc()

    xt_d = nc.declare_dram_parameter("xt", [NB, P, S], F32R, isOutput=False)
    gg_d = nc.declare_dram_parameter("gg", [NB, S, P], F32R, isOutput=False)
    dd_d = nc.declare_dram_parameter("dd", [NB, S, P], F32R, isOutput=False)
    w1_d = nc.declare_dram_parameter("w1t", [P, P], F32R, isOutput=False)
    w2_d = nc.declare_dram_parameter("w2t", [P, P], F32R, isOutput=False)
    w3_d = nc.declare_dram_parameter("w3t", [P, P], F32R, isOutput=False)
    # packed constants: b1b bf16[1024] | b2b bf16[1024] | b3c f32[8] | msk u8[128]
    cst_d = nc.declare_dram_parameter("cst", [128, 4256], U8, isOutput=False)
    out_d = nc.declare_dram_parameter("out", [NB, S, P], F32, isOutput=True)

    with tile.TileContext(nc) as tc:
        with (
            tc.tile_pool(name="wp", bufs=1) as wp,
            tc.tile_pool(name="cp", bufs=1) as cp,
            tc.tile_pool(name="xp", bufs=2) as xp,
            tc.tile_pool(name="gp", bufs=2) as gp,
            tc.tile_pool(name="qp", bufs=1) as qp,
            tc.tile_pool(name="tp", bufs=2) as tp,
            tc.tile_pool(name="ap", bufs=2) as app,
            tc.tile_pool(name="op", bufs=1) as osp,
            tc.tile_pool(name="sm", bufs=2) as sm,
            tc.tile_pool(name="ps", bufs=4, space="PSUM") as ps,
        ):
            pending_finalize = []
            for b in range(NB):
                # ---- per-batch loads ----
                u = 0
                xts = xp.tile([128, PC * S], F32R, tag="xts", name=f"xts_{b}")
                xv = xts[:].rearrange("p (c s) -> p c s", s=S)
                dv = xt_d[b].rearrange("(c p) s -> p c s", p=128)
                for hf in range(2):
                    nc.sync.dma_start(xv[:, hf * 4:(hf + 1) * 4, :],
                                      dv[:, hf * 4:(hf + 1) * 4, :])
                if b == 0:
                    # single HWDGE FIFO: small constants first (bias adds are
                    # on the critical path), then weights in use order
                    # w1 (q), w2 (k), w3 (v)
                    cst = cp.tile([128, 4256], U8, tag="cst")
                    nc.sync.dma_start(cst[:], cst_d[:])
                    ninf = cp.tile([128, 512], F32, tag="ninf")
                    nc.vector.memset(ninf[:], NEG_INF)
                    w1s = wp.tile([128, PC * P], F32R, tag="w1", name="w1s")
                    w2s = wp.tile([128, PC * P], F32R, tag="w2", name="w2s")
                    w3s = wp.tile([128, PC * P], F32R, tag="w3", name="w3s")
                    for ic in range(PC):
                        nc.sync.dma_start(
                            w1s[:, ic * P:(ic + 1) * P],
                            w1_d[ic * 128:(ic + 1) * 128, :],
                        )
                    for hh in range(2):
                        for ic in range(PC):
                            nc.sync.dma_start(
                                w2s[:, ic * P + hh * 512:
                                    ic * P + hh * 512 + 512],
                                w2_d[ic * 128:(ic + 1) * 128,
                                     hh * 512:(hh + 1) * 512],
                            )
                    gt = [gp.tile([128, P], F32R, tag=f"g{sc}",
                                  name=f"g{sc}_0") for sc in range(2)]
                    dt_ = [gp.tile([128, P], F32R, tag=f"d{sc}",
                                   name=f"d{sc}_0") for sc in range(2)]
                    for sc in range(2):
                        nc.sync.dma_start(gt[sc][:],
                                          gg_d[0, sc * 128:(sc + 1) * 128, :])
                        nc.sync.dma_start(dt_[sc][:],
                                          dd_d[0, sc * 128:(sc + 1) * 128, :])
                    for ic in range(PC):
                        nc.sync.dma_start(
                            w3s[:, ic * P:(ic + 1) * P],
                            w3_d[ic * 128:(ic + 1) * 128, :],
                        )

                if b > 0:
                    gt = [gp.tile([128, P], F32R, tag=f"g{sc}",
                                  name=f"g{sc}_{b}") for sc in range(2)]
                    dt_ = [gp.tile([128, P], F32R, tag=f"d{sc}",
                                   name=f"d{sc}_{b}") for sc in range(2)]
                    for sc in range(2):
                        nc.sync.dma_start(
                            gt[sc][:], gg_d[b, sc * 128:(sc + 1) * 128, :])
                        nc.sync.dma_start(
                            dt_[sc][:], dd_d[b, sc * 128:(sc + 1) * 128, :])

                # ---- linears: q, k in [s, e]; vT in [e, s] ----
                qt = [qp.tile([128, P], F32R, tag=f"q{sc}", name=f"q{sc}_{b}")
                      for sc in range(2)]
                kt = [qp.tile([128, P], F32R, tag=f"k{sc}", name=f"k{sc}_{b}")
                      for sc in range(2)]
                vts = qp.tile([128, PC * S], F32R, tag="vts", name=f"vts_{b}")

                for sc in range(2):
                    for h in range(2):
                        pq = ps.tile([128, 512], F32, tag="mm",
                                     name=f"pq{sc}{h}_{b}")
                        for ic in range(PC):
                            nc.tensor.matmul(
                                pq[:],
                                xts[:, ic * S + sc * 128: ic * S + sc * 128 + 128],
                                w1s[:, ic * P + h * 512: ic * P + h * 512 + 512],
                                start=(ic == 0), stop=(ic == PC - 1),
                            )
                        nc.vector.tensor_add(
                            qt[sc][:, h * 512:(h + 1) * 512], pq[:],
                            cst[:, h * 1024: h * 1024 + 1024].bitcast(BF16),
                        )
                for h in range(2):
                    for sc in range(2):
                        pk = ps.tile([128, 512], F32, tag="ob",
                                     name=f"pk{sc}{h}_{b}")
                        for ic in range(PC):
                            nc.tensor.matmul(
                                pk[:],
                                xts[:, ic * S + sc * 128: ic * S + sc * 128 + 128],
                                w2s[:, ic * P + h * 512: ic * P + h * 512 + 512],
                                start=(ic == 0), stop=(ic == PC - 1),
                            )
                        nc.vector.tensor_add(
                            kt[sc][:, h * 512:(h + 1) * 512], pk[:],
                            cst[:, 2048 + h * 1024:
                                2048 + h * 1024 + 1024].bitcast(BF16),
                        )

                if pending_finalize:
                    pending_finalize.pop(0)()

                # ---- attention scores + log_softmax + output ----
                se_all = sm.tile([128, PC], F32, tag="se_all", name=f"sea_{b}")
                m_all = sm.tile([128, PC], F32, tag="m_all", name=f"ma_{b}")
                ou = [[None, None], [None, None]]
                for sc in range(2):
                    ou[sc][0] = ps.tile([128, 512], F32, tag="ob",
                                        name=f"ou{sc}0_{b}")

                def emit_out_mms(pc, att):
                    for sc in range(2):
                        if pc == 3:
                            ou[sc][1] = ps.tile([128, 512], F32, tag="ob",
                                                name=f"ou{sc}1_{b}")
                        for h in range(2):
                            if h == 1 and pc < 3:
                                continue
                            nc.tensor.matmul(
                                ou[sc][h][:],
                                vts[:, pc * S + sc * 128: pc * S + sc * 128 + 128],
                                att[:, h * 512:(h + 1) * 512],
                                start=(pc == (0 if h == 0 else 3)),
                                stop=(pc == PC - 1),
                            )

                def emit_v_group(pc):
                    pv = ps.tile([128, 512], F32, tag="mm", name=f"pv{pc}_{b}")
                    for ic in range(PC):
                        nc.tensor.matmul(
                            pv[:, :S],
                            w3s[:, ic * P + pc * 128: ic * P + pc * 128 + 128],
                            xts[:, ic * S:(ic + 1) * S],
                            start=(ic == 0), stop=(ic == PC - 1),
                        )
                    nc.vector.tensor_scalar_add(
                        vts[:, pc * S:(pc + 1) * S], pv[:, :S],
                        cst[:, 4096 + 4 * pc: 4096 + 4 * pc + 4].bitcast(F32),
                    )

                prev = None  # (pc, att) awaiting out-matmuls
                for pc in range(PC):
                    wc = 128 * (pc + 1)          # computed width
                    wr = 512 if pc < 3 else P    # rounded (written) width
                    # score matmuls for this chunk (PE-dense, ahead of the
                    # previous chunk's out matmuls)
                    pqk = []
                    pg = []
                    for h in range(2):
                        a = h * 512
                        n = min(wc, 512 * (h + 1)) - a
                        if n <= 0:
                            pqk.append(None)
                            pg.append(None)
                            continue
                        pq_ = ps.tile([128, 512], F32, tag="mm",
                                      name=f"pqk{h}_{b}_{pc}")
                        for sc in range(2):
                            nc.tensor.matmul(
                                pq_[:, :n],
                                qt[sc][:, pc * 128:(pc + 1) * 128],
                                kt[sc][:, a: a + n],
                                start=(sc == 0), stop=(sc == 1),
                            )
                        pqk.append(pq_)
                        pg_ = ps.tile([128, 512], F32, tag="mm",
                                      name=f"pg{h}_{b}_{pc}")
                        for sc in range(2):
                            nc.tensor.matmul(
                                pg_[:, :n],
                                gt[sc][:, pc * 128:(pc + 1) * 128],
                                dt_[sc][:, a: a + n],
                                start=(sc == 0), stop=(sc == 1),
                            )
                        pg.append(pg_)

                    if prev is not None:
                        emit_v_group(prev[0])
                        emit_out_mms(*prev)

                    # t = qk * graph over [0, wc); graph staged via ACT copy
                    t = tp.tile([128, P], F32, tag="t", name=f"t_{b}_{pc}")
                    for h in range(2):
                        if pqk[h] is None:
                            continue
                        a = h * 512
                        n = min(wc, 512 * (h + 1)) - a
                        gsc = tp.tile([128, 512], F32, tag="gsc",
                                      name=f"gsc_{b}_{pc}_{h}")
                        nc.scalar.copy(gsc[:, :n], pg[h][:, :n])
                        nc.vector.tensor_tensor(
                            out=t[:, a: a + n], in0=pqk[h][:, :n],
                            in1=gsc[:, :n], op=ALU.mult,
                        )
                    # causal mask on the diagonal 128-block
                    nc.vector.copy_predicated(
                        t[:, pc * 128: pc * 128 + 128], cst[:, 4128:4256],
                        ninf[:, 0:128]
                    )
                    mxc = m_all[:, pc: pc + 1]
                    nc.vector.reduce_max(mxc, t[:, 0:wc],
                                         axis=mybir.AxisListType.X)
                    negm = sm.tile([128, 1], F32, tag="negm",
                                   name=f"negm_{b}_{pc}")
                    nc.vector.tensor_scalar_mul(negm[:], mxc, -1.0 / S)
                    att = app.tile([128, P], F32R, tag="att",
                                   name=f"att_{b}_{pc}")
                    # exp for the row sums only (att is scratch here)
                    nc.scalar.activation(
                        att[:, 0:wc], t[:, 0:wc], AF.Exp, bias=negm[:],
                        scale=1.0 / S, accum_out=se_all[:, pc: pc + 1],
                    )
                    # att = t/S (finite part); -inf tail
                    nc.vector.tensor_scalar_mul(att[:, 0:wc], t[:, 0:wc],
                                                1.0 / S)
                    if wr > wc:
                        nc.vector.tensor_copy(att[:, wc:wr], ninf[:, 0:wr - wc])
                    prev = (pc, att)

                emit_v_group(prev[0])
                emit_out_mms(*prev)

                # ---- deferred finalize: c = m/S + ln(se); rank-1 fold ----
                def make_finalize(b, u, se_all, m_all, vts, ou):
                    def finalize():
                        lna = sm.tile([128, PC], F32, tag="lna",
                                      name=f"lna_{b}")
                        nc.scalar.activation(lna[:], se_all[:], AF.Ln)
                        c_all = sm.tile([128, PC], F32R, tag="c_all",
                                        name=f"ca_{b}")
                        nc.vector.tensor_scalar(
                            out=c_all[:], in0=m_all[:], scalar1=1.0 / S,
                            scalar2=None, op0=ALU.mult,
                        )
                        nc.vector.tensor_tensor(out=c_all[:], in0=c_all[:],
                                                in1=lna[:], op=ALU.add)
                        # fp32r matmul needs even moving-free-dim: duplicate
                        # c into even/odd cols of c2 (col 1 result unused)
                        c2 = sm.tile([128, 2 * PC], F32R, tag="c2",
                                     name=f"c2_{b}")
                        c2v = c2[:].rearrange("p (a two) -> p two a", two=2)
                        nc.vector.tensor_copy(c2v[:, 0, :], c_all[:])
                        nc.vector.tensor_copy(c2v[:, 1, :], c_all[:])
                        vcs = sm.tile([128, 2], F32, tag="vcs",
                                      name=f"vcs_{b}")
                        for sc in range(2):
                            pvc = ps.tile([128, 512], F32, tag="mm",
                                          name=f"pvc{sc}_{b}")
                            for pc in range(PC):
                                nc.tensor.matmul(
                                    pvc[:, 0:2],
                                    vts[:, pc * S + sc * 128:
                                        pc * S + sc * 128 + 128],
                                    c2[:, 2 * pc: 2 * pc + 2],
                                    start=(pc == 0), stop=(pc == PC - 1),
                                )
                            nc.vector.tensor_copy(vcs[:, sc: sc + 1],
                                                  pvc[:, 0:1])
                            osb = osp.tile([128, P], F32, tag="osb",
                                           name=f"osb{sc}_{b}")
                            for h in range(2):
                                nc.vector.tensor_scalar_sub(
                                    osb[:, h * 512:(h + 1) * 512],
                                    ou[sc][h][:], vcs[:, sc: sc + 1],
                                )
                            nc.sync.dma_start(
                                out_d[b, sc * 128:(sc + 1) * 128, :], osb[:]
                            )
                    return finalize

                pending_finalize.append(make_finalize(b, u, se_all, m_all, vts, ou))

            while pending_finalize:
                pending_finalize.pop(0)()

    nc.compile()
    _prog_cache["nc"] = nc
    return nc


def kernel(user_feature, global_graph_feature, global_dist_feature,
           W1, b1, W2, b2, W3, b3):
    uf = np.ascontiguousarray(np.asarray(user_feature, dtype=np.float32))
    gg = np.ascontiguousarray(np.asarray(global_graph_feature, dtype=np.float32))
    dd = np.ascontiguousarray(np.asarray(global_dist_feature, dtype=np.float32))
    W1 = np.asarray(W1, dtype=np.float32)
    W2 = np.asarray(W2, dtype=np.float32)
    W3 = np.asarray(W3, dtype=np.float32)
    b1 = np.asarray(b1, dtype=np.float32)
    b2 = np.asarray(b2, dtype=np.float32)
    b3 = np.asarray(b3, dtype=np.float32)

    nc = build_program()

    xt = np.ascontiguousarray(np.swapaxes(uf, 1, 2))          # [B, P, S]
    w1t = np.ascontiguousarray(W1.T)
    w2t = np.ascontiguousarray(W2.T)
    w3t = np.ascontiguousarray(W3.T)
    import ml_dtypes
    b1b = np.broadcast_to(b1, (128, P)).astype(ml_dtypes.bfloat16)
    b2b = np.broadcast_to(b2, (128, P)).astype(ml_dtypes.bfloat16)
    b3c = np.ascontiguousarray(b3.reshape(PC, 128).T)          # [128, PC]
    j = np.arange(128)[None, :]
    r = np.arange(128)[:, None]
    msk = (j > r).astype(np.uint8)                             # diag block mask
    cst = np.zeros((128, 4256), np.uint8)
    cst[:, 0:2048] = np.ascontiguousarray(b1b).view(np.uint8)
    cst[:, 2048:4096] = np.ascontiguousarray(b2b).view(np.uint8)
    cst[:, 4096:4128] = b3c.view(np.uint8)
    cst[:, 4128:4256] = msk

    in_maps = []
    for c in range(NCORES):
        sl = slice(c * NB, (c + 1) * NB)
        in_maps.append({
            "xt": xt[sl], "gg": gg[sl], "dd": dd[sl],
            "w1t": w1t, "w2t": w2t, "w3t": w3t, "cst": cst,
        })

    res = run_bass_kernel_spmd(nc, in_maps, list(range(NCORES)))
    out = np.empty((B, S, P), np.float32)
    for c in range(NCORES):
        out[c * NB:(c + 1) * NB] = res.results[c]["out"]
    return out
